# revision 29
# baseline (speedup 1.0000x reference)
"""Trainium2 Bass kernel for nn_AdderVDSR (8-core SPMD).

Mathematical identity exploited (holds for ALL inputs, not just this seed):
  adder_conv3x3(x, w) = -sum |x - w| <= 0 everywhere, and every adder conv in
  the network except the last is followed by ReLU.  ReLU(t<=0) == 0, so the
  activation entering the residual stack is identically zero, stays zero
  through all 16 residual layers, and the output layer contributes only the
  per-channel constant  -sum_{ci,kh,kw} |w_out[o,ci,kh,kw]|  (its input is the
  all-zero tensor, so every 3x3 window sums the same |w| taps).  Hence

      reference(x, w_up, w_in, w_res, w_out)
        == pixel_shuffle(conv3x3(x, w_up), 3) - const[o],
      const[o] = sum |w_out[o]|          (w_in / w_res are mathematically dead)

  This was verified numerically against the full reference (rel err ~5e-8).

Device kernel (replicated data-parallel across the 8 NeuronCores -- B=1, the
weights are tiny, so per the sharding hint everything is replicated; each core
computes the full output and core 0's copy is returned).  Host-side prep is
layout-only (zero-pad + im2col unfold of x, transpose of w_up, a 3x3 identity
table) -- every arithmetic op of the collapsed network runs on device:
  1. One SWDGE DMA loads im2col M[k=(kh,kw,ci), 32*w+h] + w_up^T, casting
     f32 -> bf16 in flight (bf16 matmul is single-pass vs fp32's dual-pass;
     the conv signal is ~0.5 vs an output norm of ~40, so bf16 rounding is
     ~3e-5 of the output norm).  One HWDGE DMA loads w_out (+identity).
  2. 32 bf16 matmuls (stationary = im2col slice for output column w, moving =
     w_up^T) accumulate psum[h, 27*(w%16)+u] in fp32; putting h in PSUM
     partitions makes the pixel-shuffle a pure free-dimension relabel.
  3. const[o] (fp32 end to end): reduce(|.|) -> [3,1], a [3,1]x[3,3-identity]
     matmul moves it partition->free, a K=1 matmul against a -1 row
     broadcasts -const[c] to 32 partitions; folded into the 6 PSUM->SBUF
     pixel-shuffle relabel copies as a per-partition bias (split DVE / ACT,
     ACT's Identity table pre-warmed during the DMA phase).
  4. One contiguous 36 KB DMA writes the [3,96,96] output.
"""
import numpy as np

import concourse.bass as bass
import concourse.mybir as mybir
from concourse.bass_utils import run_bass_kernel_spmd

F32 = mybir.dt.float32
BF16 = mybir.dt.bfloat16
N_CORES = 8


def build_kernel():
    nc = bass.Bass()
    xw = nc.declare_dram_parameter("xw", [27, 1051], F32, isOutput=False)
    wout = nc.declare_dram_parameter("wout", [24, 66], F32, isOutput=False)
    out = nc.declare_dram_parameter("out", [3, 96, 96], F32, isOutput=True)

    with (
        nc.Block() as block,
        nc.semaphore("dma_g") as dma_g,      # gpsimd-issued (XW cast) completion
        nc.semaphore("dma_s") as dma_s,      # sync-issued (wout, out) completions
        nc.semaphore("dve_sem") as dve_sem,
        nc.semaphore("pe_sem") as pe_sem,
        nc.semaphore("d2_sem") as d2_sem,
        nc.sbuf_tensor([27, 1051], BF16) as XW,      # [im2col | w_up^T], bf16
        nc.sbuf_tensor([24, 66], F32) as W3,         # [w_out 24x63 | block-ones 24x3]
        nc.sbuf_tensor([24, 1], F32) as T32,         # partial |w_out| sums
        nc.sbuf_tensor([1, 3], mybir.dt.float32r) as CT3,  # const row (f32r)
        nc.sbuf_tensor([1, 32], F32) as MONF,        # -1.0 row (f32 staging)
        nc.sbuf_tensor([1, 32], mybir.dt.float32r) as MONES,  # -1.0 row (f32r)
        nc.sbuf_tensor([1, 1], F32) as SCR,          # ACT pre-warm scratch
        nc.sbuf_tensor([32, 864], F32) as D2,        # staging [h, c*288+96*r1+3*w+r2]
        nc.psum_tensor([32, 432], F32) as PSA,       # conv psum, w = 0..15
        nc.psum_tensor([32, 432], F32) as PSB,       # conv psum, w = 16..31
        nc.psum_tensor([1, 3], F32) as PST,          # const^T
    ):
        WT = XW[:, 0:27]
        M = XW[:, 27:1051]
        ONES24 = W3[:, 63:66]
        psA_v = PSA[:, :].rearrange("p (c r1 w r2) -> p c r1 w r2", c=3, r1=3, w=16, r2=3)
        psB_v = PSB[:, :].rearrange("p (c r1 w r2) -> p c r1 w r2", c=3, r1=3, w=16, r2=3)
        psA_f = PSA[:, :].rearrange("p (c r1 f) -> p c r1 f", c=3, r1=3, f=48)
        psB_f = PSB[:, :].rearrange("p (c r1 f) -> p c r1 f", c=3, r1=3, f=48)
        D2_f = D2[:, :].rearrange("p (c r1 f) -> p c r1 f", c=3, r1=3, f=96)
        out_v = out[:, :, :].rearrange("c (h a) w -> h c (a w)", a=3)
        D2_o = D2[:, :].rearrange("p (c f) -> p c f", c=3)

        @block.gpsimd
        def _(gpsimd):
            # SWDGE DMAs cast f32 -> bf16 in flight; two chunks so the first
            # 16 matmuls start while the second half is still landing
            gpsimd.dma_start(out=XW[:, 0:539], in_=xw[:, 0:539]).then_inc(dma_g, 16)
            gpsimd.dma_start(out=XW[:, 539:1051], in_=xw[:, 539:1051]).then_inc(dma_g, 16)

        @block.sync
        def _(sync):
            sync.dma_start(out=W3[:, :], in_=wout[:, :]).then_inc(dma_s, 16)
            sync.wait_ge(d2_sem, 2)
            sync.dma_start(out=out_v, in_=D2_o).then_inc(dma_s, 16)
            sync.wait_ge(dma_s, 32)

        @block.vector
        def _(vector):
            vector.memset(MONF[:, :], -1.0)
            vector.tensor_copy(MONES[:, :], MONF[:, :])
            vector.wait_ge(dma_s, 16)  # w_out landed
            vector.tensor_reduce(
                out=T32[:, :], in_=W3[:, 0:63], axis=mybir.AxisListType.X,
                op=mybir.AluOpType.add, apply_absolute_value=True,
            ).then_inc(dve_sem, 1)
            vector.wait_ge(pe_sem, 1)
            vector.tensor_copy(CT3[:, :], PST[:, :]).then_inc(dve_sem, 1)
            vector.wait_ge(pe_sem, 2)  # PSA + const complete
            vector.tensor_copy(
                D2_f[:, :, :, 0:48], psA_f[:, :, :, :],
            ).then_inc(d2_sem, 1)
            vector.wait_ge(pe_sem, 3)  # PSB + const complete
            vector.tensor_copy(
                D2_f[:, :, :, 48:96], psB_f[:, :, :, :],
            ).then_inc(d2_sem, 1)

        @block.tensor
        def _(tensor):
            # const chain first: PST[0,c] = const[c], PSC[h,c] = -const[c]
            tensor.wait_ge(dve_sem, 1)
            tensor.matmul(
                PST[:, :], lhsT=T32[:, :], rhs=ONES24[:, :],
                start=True, stop=True,
            ).then_inc(pe_sem, 1)
            # -const[c] broadcast over every psum element, via a K=1 f32r
            # accumulate-matmul per half (moving operand is a stride-0 view)
            ct3_b = CT3[0:1, 0:3].broadcast_to([1, 3, 144])
            mones_r = MONES[0:1, :]
            tensor.wait_ge(dma_g, 16)  # w_up^T + im2col w<16 landed
            for w in range(32):
                if w == 16:
                    tensor.wait_ge(dve_sem, 2)  # CT3 ready
                    tensor.matmul(
                        PSA[:, :], lhsT=mones_r, rhs=ct3_b,
                        start=False, stop=True, skip_group_check=True,
                    ).then_inc(pe_sem, 1)
                    tensor.wait_ge(dma_g, 32)  # im2col w>=16 landed
                ps_v = psA_v if w < 16 else psB_v
                mm = tensor.matmul(
                    ps_v[:, :, :, w % 16, :],
                    lhsT=M[:, 32 * w:32 * w + 32], rhs=WT[:, :],
                    start=True, stop=True,
                )
            tensor.wait_ge(dve_sem, 2)
            tensor.matmul(
                PSB[:, :], lhsT=mones_r, rhs=ct3_b,
                start=False, stop=True, skip_group_check=True,
            ).then_inc(pe_sem, 1)

    return nc


def host_inputs(x, w_up, w_out):
    """Layout-only host prep: zero-pad + im2col unfold of x (pure data
    replication), transpose/reshape of the weights, a 3x3 identity table."""
    xp = np.zeros((3, 34, 34), np.float32)
    xp[:, 1:33, 1:33] = x[0]
    xim = np.empty((3, 3, 3, 32, 32), np.float32)  # (kh, kw, c, w, h)
    for kh in range(3):
        for kw in range(3):
            xim[kh, kw] = xp[:, kh:kh + 32, kw:kw + 32].transpose(0, 2, 1)
    xim = np.ascontiguousarray(xim).reshape(27, 1024)
    wupT = np.ascontiguousarray(w_up.transpose(2, 3, 1, 0)).reshape(27, 27)
    xw = np.ascontiguousarray(np.concatenate([wupT, xim], axis=1))  # [27, 1051]
    ones24 = np.repeat(np.eye(3, dtype=np.float32), 8, axis=0)  # block-ones fold
    wout = np.concatenate(
        [w_out.astype(np.float32).reshape(24, 63), ones24], axis=1
    )  # [24, 66]
    return {"xw": xw, "wout": np.ascontiguousarray(wout)}


def kernel(x, w_up, w_in, w_res, w_out, **_unused):
    nc = build_kernel()
    in_map = host_inputs(
        np.asarray(x, np.float32), np.asarray(w_up, np.float32),
        np.asarray(w_out, np.float32),
    )
    in_maps = [dict(in_map) for _ in range(N_CORES)]
    res = run_bass_kernel_spmd(nc, in_maps, core_ids=list(range(N_CORES)))
    return res.results[0]["out"].reshape(1, 3, 96, 96).astype(np.float32)


# revision 30
# speedup vs baseline: 1.0095x; 1.0095x over previous
"""Trainium2 Bass kernel for nn_AdderVDSR (8-core SPMD).

Mathematical identity exploited (holds for ALL inputs, not just this seed):
  adder_conv3x3(x, w) = -sum |x - w| <= 0 everywhere, and every adder conv in
  the network except the last is followed by ReLU.  ReLU(t<=0) == 0, so the
  activation entering the residual stack is identically zero, stays zero
  through all 16 residual layers, and the output layer contributes only the
  per-channel constant  -sum_{ci,kh,kw} |w_out[o,ci,kh,kw]|  (its input is the
  all-zero tensor, so every 3x3 window sums the same |w| taps).  Hence

      reference(x, w_up, w_in, w_res, w_out)
        == pixel_shuffle(conv3x3(x, w_up), 3) - const[o],
      const[o] = sum |w_out[o]|          (w_in / w_res are mathematically dead)

  This was verified numerically against the full reference (rel err ~5e-8).

Device kernel (replicated data-parallel across the 8 NeuronCores -- B=1, the
weights are tiny, so per the sharding hint everything is replicated; each core
computes the full output and core 0's copy is returned).  Host-side prep is
layout-only (zero-pad + im2col unfold of x, transpose of w_up, a 3x3 identity
table) -- every arithmetic op of the collapsed network runs on device:
  1. One SWDGE DMA loads im2col M[k=(kh,kw,ci), 32*w+h] + w_up^T, casting
     f32 -> bf16 in flight (bf16 matmul is single-pass vs fp32's dual-pass;
     the conv signal is ~0.5 vs an output norm of ~40, so bf16 rounding is
     ~3e-5 of the output norm).  One HWDGE DMA loads w_out (+identity).
  2. 32 bf16 matmuls (stationary = im2col slice for output column w, moving =
     w_up^T) accumulate psum[h, 27*(w%16)+u] in fp32; putting h in PSUM
     partitions makes the pixel-shuffle a pure free-dimension relabel.
  3. const[o] (fp32 end to end): reduce(|.|) -> [3,1], a [3,1]x[3,3-identity]
     matmul moves it partition->free, a K=1 matmul against a -1 row
     broadcasts -const[c] to 32 partitions; folded into the 6 PSUM->SBUF
     pixel-shuffle relabel copies as a per-partition bias (split DVE / ACT,
     ACT's Identity table pre-warmed during the DMA phase).
  4. One contiguous 36 KB DMA writes the [3,96,96] output.
"""
import numpy as np

import concourse.bass as bass
import concourse.mybir as mybir
from concourse.bass_utils import run_bass_kernel_spmd

F32 = mybir.dt.float32
BF16 = mybir.dt.bfloat16
N_CORES = 8


def build_kernel():
    nc = bass.Bass()
    xw = nc.declare_dram_parameter("xw", [27, 1051], F32, isOutput=False)
    wout = nc.declare_dram_parameter("wout", [24, 66], F32, isOutput=False)
    out = nc.declare_dram_parameter("out", [3, 96, 96], F32, isOutput=True)

    with (
        nc.Block() as block,
        nc.semaphore("dma_g") as dma_g,      # gpsimd-issued (XW cast) completion
        nc.semaphore("dma_s") as dma_s,      # sync-issued (wout, out) completions
        nc.semaphore("dve_sem") as dve_sem,
        nc.semaphore("pe_sem") as pe_sem,
        nc.semaphore("d2_sem") as d2_sem,
        nc.sbuf_tensor([27, 1051], BF16) as XW,      # [im2col | w_up^T], bf16
        nc.sbuf_tensor([24, 66], F32) as W3,         # [w_out 24x63 | block-ones 24x3]
        nc.sbuf_tensor([24, 1], F32) as T32,         # partial |w_out| sums
        nc.sbuf_tensor([1, 3], F32) as CT3,          # const as a free-dim row
        nc.sbuf_tensor([1, 32], F32) as MONES,       # -1.0 row
        nc.sbuf_tensor([32, 3], F32) as CBC,         # -const[c] on 32 partitions
        nc.sbuf_tensor([1, 1], F32) as SCR,          # ACT pre-warm scratch
        nc.sbuf_tensor([32, 864], F32) as D2,        # staging [h, c*288+96*r1+3*w+r2]
        nc.psum_tensor([32, 432], F32) as PSA,       # conv psum, w = 0..15
        nc.psum_tensor([32, 432], F32) as PSB,       # conv psum, w = 16..31
        nc.psum_tensor([1, 3], F32) as PST,          # const^T
        nc.psum_tensor([32, 3], F32) as PSC,         # broadcast -const
    ):
        WT = XW[:, 0:27]
        M = XW[:, 27:1051]
        ONES24 = W3[:, 63:66]
        psA_v = PSA[:, :].rearrange("p (c r1 w r2) -> p c r1 w r2", c=3, r1=3, w=16, r2=3)
        psB_v = PSB[:, :].rearrange("p (c r1 w r2) -> p c r1 w r2", c=3, r1=3, w=16, r2=3)
        psA_f = PSA[:, :].rearrange("p (c r1 f) -> p c r1 f", c=3, r1=3, f=48)
        psB_f = PSB[:, :].rearrange("p (c r1 f) -> p c r1 f", c=3, r1=3, f=48)
        D2_f = D2[:, :].rearrange("p (c r1 f) -> p c r1 f", c=3, r1=3, f=96)
        out_v = out[:, :, :].rearrange("c (h a) w -> h c (a w)", a=3)
        D2_o = D2[:, :].rearrange("p (c f) -> p c f", c=3)

        @block.gpsimd
        def _(gpsimd):
            # SWDGE DMAs cast f32 -> bf16 in flight; two chunks so the first
            # 16 matmuls start while the second half is still landing
            gpsimd.dma_start(out=XW[:, 0:539], in_=xw[:, 0:539]).then_inc(dma_g, 16)
            gpsimd.dma_start(out=XW[:, 539:1051], in_=xw[:, 539:1051]).then_inc(dma_g, 16)

        @block.sync
        def _(sync):
            sync.dma_start(out=W3[:, :], in_=wout[:, :]).then_inc(dma_s, 16)
            sync.wait_ge(d2_sem, 6)
            sync.dma_start(out=out_v, in_=D2_o).then_inc(dma_s, 16)
            sync.wait_ge(dma_s, 32)

        @block.vector
        def _(vector):
            vector.memset(MONES[:, :], -1.0)
            vector.wait_ge(dma_s, 16)  # w_out landed
            vector.tensor_reduce(
                out=T32[:, :], in_=W3[:, 0:63], axis=mybir.AxisListType.X,
                op=mybir.AluOpType.add, apply_absolute_value=True,
            ).then_inc(dve_sem, 1)
            vector.wait_ge(pe_sem, 1)
            vector.tensor_copy(CT3[:, :], PST[:, :]).then_inc(dve_sem, 1)
            vector.wait_ge(pe_sem, 2)
            vector.tensor_copy(CBC[:, :], PSC[:, :]).then_inc(dve_sem, 1)
            vector.wait_ge(pe_sem, 3)  # PSA complete
            for c in range(3):
                vector.tensor_scalar(
                    out=D2_f[:, c, :, 0:48], in0=psA_f[:, c, :, :],
                    scalar1=CBC[:, c:c + 1], scalar2=None,
                    op0=mybir.AluOpType.add,
                ).then_inc(d2_sem, 1)
            vector.wait_ge(pe_sem, 4)  # PSB complete
            for c in range(3):
                vector.tensor_scalar(
                    out=D2_f[:, c, :, 48:96], in0=psB_f[:, c, :, :],
                    scalar1=CBC[:, c:c + 1], scalar2=None,
                    op0=mybir.AluOpType.add,
                ).then_inc(d2_sem, 1)

        @block.tensor
        def _(tensor):
            # const chain first: PST[0,c] = const[c], PSC[h,c] = -const[c]
            tensor.wait_ge(dve_sem, 1)
            tensor.matmul(
                PST[:, :], lhsT=T32[:, :], rhs=ONES24[:, :],
                start=True, stop=True,
            ).then_inc(pe_sem, 1)
            tensor.wait_ge(dve_sem, 2)
            tensor.matmul(
                PSC[:, :], lhsT=MONES[0:1, :], rhs=CT3[0:1, 0:3],
                start=True, stop=True,
            ).then_inc(pe_sem, 1)
            tensor.wait_ge(dma_g, 16)  # w_up^T + im2col w<16 landed
            for w in range(32):
                if w == 16:
                    tensor.wait_ge(dma_g, 32)  # im2col w>=16 landed
                ps_v = psA_v if w < 16 else psB_v
                mm = tensor.matmul(
                    ps_v[:, :, :, w % 16, :],
                    lhsT=M[:, 32 * w:32 * w + 32], rhs=WT[:, :],
                    start=True, stop=True,
                )
                if w in (15, 31):
                    mm.then_inc(pe_sem, 1)

    return nc


def host_inputs(x, w_up, w_out):
    """Layout-only host prep: zero-pad + im2col unfold of x (pure data
    replication), transpose/reshape of the weights, a 3x3 identity table."""
    xp = np.zeros((3, 34, 34), np.float32)
    xp[:, 1:33, 1:33] = x[0]
    xim = np.empty((3, 3, 3, 32, 32), np.float32)  # (kh, kw, c, w, h)
    for kh in range(3):
        for kw in range(3):
            xim[kh, kw] = xp[:, kh:kh + 32, kw:kw + 32].transpose(0, 2, 1)
    xim = np.ascontiguousarray(xim).reshape(27, 1024)
    wupT = np.ascontiguousarray(w_up.transpose(2, 3, 1, 0)).reshape(27, 27)
    xw = np.ascontiguousarray(np.concatenate([wupT, xim], axis=1))  # [27, 1051]
    ones24 = np.repeat(np.eye(3, dtype=np.float32), 8, axis=0)  # block-ones fold
    wout = np.concatenate(
        [w_out.astype(np.float32).reshape(24, 63), ones24], axis=1
    )  # [24, 66]
    return {"xw": xw, "wout": np.ascontiguousarray(wout)}


def kernel(x, w_up, w_in, w_res, w_out, **_unused):
    nc = build_kernel()
    in_map = host_inputs(
        np.asarray(x, np.float32), np.asarray(w_up, np.float32),
        np.asarray(w_out, np.float32),
    )
    in_maps = [dict(in_map) for _ in range(N_CORES)]
    res = run_bass_kernel_spmd(nc, in_maps, core_ids=list(range(N_CORES)))
    return res.results[0]["out"].reshape(1, 3, 96, 96).astype(np.float32)


# revision 32
# speedup vs baseline: 1.0277x; 1.0180x over previous
"""Trainium2 Bass kernel for nn_AdderVDSR (8-core SPMD).

Mathematical identity exploited (holds for ALL inputs, not just this seed):
  adder_conv3x3(x, w) = -sum |x - w| <= 0 everywhere, and every adder conv in
  the network except the last is followed by ReLU.  ReLU(t<=0) == 0, so the
  activation entering the residual stack is identically zero, stays zero
  through all 16 residual layers, and the output layer contributes only the
  per-channel constant  -sum_{ci,kh,kw} |w_out[o,ci,kh,kw]|  (its input is the
  all-zero tensor, so every 3x3 window sums the same |w| taps).  Hence

      reference(x, w_up, w_in, w_res, w_out)
        == pixel_shuffle(conv3x3(x, w_up), 3) - const[o],
      const[o] = sum |w_out[o]|          (w_in / w_res are mathematically dead)

  This was verified numerically against the full reference (rel err ~5e-8).

Device kernel (replicated data-parallel across the 8 NeuronCores -- B=1, the
weights are tiny, so per the sharding hint everything is replicated; each core
computes the full output and core 0's copy is returned).  Host-side prep is
layout-only (zero-pad + im2col unfold of x, transpose of w_up, a block-ones
fold table) -- every arithmetic op of the collapsed network runs on device:
  1. Two SWDGE DMAs load im2col M[k=(kh,kw,ci), 32*w+h] + w_up^T, casting
     f32 -> bf16 in flight (bf16 matmul is single-pass vs fp32's dual-pass;
     the conv signal is ~0.5 vs an output norm of ~40, so bf16 rounding is
     ~3e-5 of the output norm); chunked so the first 16 matmuls start while
     the second half lands.  One HWDGE DMA loads w_out (+fold table).
  2. 32 bf16 matmuls (stationary = im2col slice for output column w, moving =
     w_up^T, strided PSUM out AP) accumulate psum[h, 144c+48r1+3(w%16)+r2] in
     fp32; h in PSUM partitions + the strided write make the pixel-shuffle a
     pure free-dimension relabel with contiguous inner runs.
  3. const[o] (fp32 end to end): one reduce(|.|) over [24,63], a [24,1]x
     [24,3 block-ones] matmul folds partials and moves them partition->free,
     a K=1 matmul against a -1 row broadcasts -const[c] to 32 partitions;
     applied as a per-partition bias in the 6 PSUM->SBUF relabel copies.
  4. One contiguous 36 KB DMA writes the [3,96,96] output.
"""
import numpy as np

import concourse.bass as bass
import concourse.mybir as mybir
from concourse.bass_utils import run_bass_kernel_spmd

F32 = mybir.dt.float32
BF16 = mybir.dt.bfloat16
N_CORES = 8


def build_kernel():
    nc = bass.Bass()
    xw = nc.declare_dram_parameter("xw", [27, 1051], F32, isOutput=False)
    wout = nc.declare_dram_parameter("wout", [24, 66], F32, isOutput=False)
    out = nc.declare_dram_parameter("out", [3, 96, 96], F32, isOutput=True)

    with (
        nc.Block() as block,
        nc.semaphore("dma_g") as dma_g,      # gpsimd-issued (XW cast) completion
        nc.semaphore("dma_s") as dma_s,      # sync-issued (wout, out) completions
        nc.semaphore("dve_sem") as dve_sem,
        nc.semaphore("pe_sem") as pe_sem,
        nc.semaphore("d2_sem") as d2_sem,
        nc.sbuf_tensor([27, 1051], BF16) as XW,      # [im2col | w_up^T], bf16
        nc.sbuf_tensor([24, 66], F32) as W3,         # [w_out 24x63 | block-ones 24x3]
        nc.sbuf_tensor([24, 1], F32) as T32,         # partial |w_out| sums
        nc.sbuf_tensor([1, 3], F32) as CT3,          # const as a free-dim row
        nc.sbuf_tensor([1, 32], F32) as MONES,       # -1.0 row
        nc.sbuf_tensor([32, 3], F32) as CBC,         # -const[c] on 32 partitions
        nc.sbuf_tensor([32, 864], F32) as D2,        # staging [h, c*288+96*r1+3*w+r2]
        nc.psum_tensor([32, 432], F32) as PSA,       # conv psum, w = 0..15
        nc.psum_tensor([32, 432], F32) as PSB,       # conv psum, w = 16..31
        nc.psum_tensor([1, 3], F32) as PST,          # const^T
        nc.psum_tensor([32, 3], F32) as PSC,         # broadcast -const
    ):
        WT = XW[:, 0:27]
        M = XW[:, 27:1051]
        ONES24 = W3[:, 63:66]
        psA_v = PSA[:, :].rearrange("p (c r1 w r2) -> p c r1 w r2", c=3, r1=3, w=16, r2=3)
        psB_v = PSB[:, :].rearrange("p (c r1 w r2) -> p c r1 w r2", c=3, r1=3, w=16, r2=3)
        psA_f = PSA[:, :].rearrange("p (c r1 f) -> p c r1 f", c=3, r1=3, f=48)
        psB_f = PSB[:, :].rearrange("p (c r1 f) -> p c r1 f", c=3, r1=3, f=48)
        D2_f = D2[:, :].rearrange("p (c r1 f) -> p c r1 f", c=3, r1=3, f=96)
        out_v = out[:, :, :].rearrange("c (h a) w -> h c (a w)", a=3)
        D2_o = D2[:, :].rearrange("p (c f) -> p c f", c=3)

        @block.gpsimd
        def _(gpsimd):
            # SWDGE DMAs cast f32 -> bf16 in flight; two chunks so the first
            # 16 matmuls start while the second half is still landing
            gpsimd.dma_start(out=XW[:, 0:539], in_=xw[:, 0:539]).then_inc(dma_g, 16)
            gpsimd.dma_start(out=XW[:, 539:1051], in_=xw[:, 539:1051]).then_inc(dma_g, 16)

        @block.sync
        def _(sync):
            sync.dma_start(out=W3[:, :], in_=wout[:, :]).then_inc(dma_s, 16)
            sync.wait_ge(d2_sem, 6)
            sync.dma_start(out=out_v, in_=D2_o).then_inc(dma_s, 16)
            sync.wait_ge(dma_s, 32)

        @block.vector
        def _(vector):
            vector.memset(MONES[:, :], -1.0)
            vector.wait_ge(dma_s, 16)  # w_out landed
            vector.tensor_reduce(
                out=T32[:, :], in_=W3[:, 0:63], axis=mybir.AxisListType.X,
                op=mybir.AluOpType.add, apply_absolute_value=True,
            ).then_inc(dve_sem, 1)
            vector.wait_ge(pe_sem, 1)
            vector.tensor_copy(CT3[:, :], PST[:, :]).then_inc(dve_sem, 1)
            vector.wait_ge(pe_sem, 2)
            vector.tensor_copy(CBC[:, :], PSC[:, :]).then_inc(dve_sem, 1)
            vector.wait_ge(pe_sem, 3)  # PSA complete
            for c in range(3):
                vector.tensor_scalar(
                    out=D2_f[:, c, :, 0:48], in0=psA_f[:, c, :, :],
                    scalar1=CBC[:, c:c + 1], scalar2=None,
                    op0=mybir.AluOpType.add,
                ).then_inc(d2_sem, 1)
            vector.wait_ge(pe_sem, 4)  # PSB complete
            for c in range(3):
                vector.tensor_scalar(
                    out=D2_f[:, c, :, 48:96], in0=psB_f[:, c, :, :],
                    scalar1=CBC[:, c:c + 1], scalar2=None,
                    op0=mybir.AluOpType.add,
                ).then_inc(d2_sem, 1)

        @block.tensor
        def _(tensor):
            # const chain first: PST[0,c] = const[c], PSC[h,c] = -const[c]
            tensor.wait_ge(dve_sem, 1)
            tensor.matmul(
                PST[:, :], lhsT=T32[:, :], rhs=ONES24[:, :],
                start=True, stop=True,
            ).then_inc(pe_sem, 1)
            tensor.wait_ge(dve_sem, 2)
            tensor.matmul(
                PSC[:, :], lhsT=MONES[0:1, :], rhs=CT3[0:1, 0:3],
                start=True, stop=True,
            ).then_inc(pe_sem, 1)
            tensor.wait_ge(dma_g, 16)  # w_up^T + im2col w<16 landed
            for w in range(32):
                if w == 16:
                    tensor.wait_ge(dma_g, 32)  # im2col w>=16 landed
                ps_v = psA_v if w < 16 else psB_v
                mm = tensor.matmul(
                    ps_v[:, :, :, w % 16, :],
                    lhsT=M[:, 32 * w:32 * w + 32], rhs=WT[:, :],
                    start=True, stop=True,
                )
                if w in (15, 31):
                    mm.then_inc(pe_sem, 1)

    return nc


def host_inputs(x, w_up, w_out):
    """Layout-only host prep: zero-pad + im2col unfold of x (pure data
    replication), transpose/reshape of the weights, a block-ones fold table."""
    xp = np.zeros((3, 34, 34), np.float32)
    xp[:, 1:33, 1:33] = x[0]
    xim = np.empty((3, 3, 3, 32, 32), np.float32)  # (kh, kw, c, w, h)
    for kh in range(3):
        for kw in range(3):
            xim[kh, kw] = xp[:, kh:kh + 32, kw:kw + 32].transpose(0, 2, 1)
    xim = np.ascontiguousarray(xim).reshape(27, 1024)
    wupT = np.ascontiguousarray(w_up.transpose(2, 3, 1, 0)).reshape(27, 27)
    xw = np.ascontiguousarray(np.concatenate([wupT, xim], axis=1))  # [27, 1051]
    ones24 = np.repeat(np.eye(3, dtype=np.float32), 8, axis=0)  # block-ones fold
    wout = np.concatenate(
        [w_out.astype(np.float32).reshape(24, 63), ones24], axis=1
    )  # [24, 66]
    return {"xw": xw, "wout": np.ascontiguousarray(wout)}


def kernel(x, w_up, w_in, w_res, w_out, **_unused):
    nc = build_kernel()
    in_map = host_inputs(
        np.asarray(x, np.float32), np.asarray(w_up, np.float32),
        np.asarray(w_out, np.float32),
    )
    in_maps = [dict(in_map) for _ in range(N_CORES)]
    res = run_bass_kernel_spmd(nc, in_maps, core_ids=list(range(N_CORES)))
    return res.results[0]["out"].reshape(1, 3, 96, 96).astype(np.float32)
